# Initial kernel scaffold
#
"""GPT2 block kernel for 8 TRN2 NeuronCores (Bass/Tile, SPMD).

Sharding: the 4096 rows (batch*seq) are split 8 ways -> 512 rows/core
(4 cores per batch element). Each core redundantly computes K,V for its
batch, then causal attention for its own 512 query rows against all
2048 keys, then proj/LN2/MLP for its own rows only. Zero collectives.

All tensors are kept in a "transposed" layout (feature dim on SBUF
partitions, token dim on the free axis) so no on-device transposes are
needed. The host transposes inputs/outputs and rotates each core's key
order so its own rows are always key-quarter 0 (attention is
permutation-invariant under a matching mask, which is passed as data).
"""

import numpy as np
import sys

sys.path.insert(0, "/opt/trn_rl_repo")

import concourse.bacc as bacc
import concourse.mybir as mybir
import concourse.tile as tile
from concourse import bass_utils

dt = mybir.dt
F = mybir.ActivationFunctionType
Alu = mybir.AluOpType

D = 1024
S = 2048
Q = 512        # own rows per core
H = 16
HD = 64
INNER = 4096
P = 128
DC = D // P    # 8
IC = INNER // P  # 32
EPS = 1e-5
NQT = 4        # key quarters
KQ = S // NQT  # 512 keys per quarter
NKT = KQ // P  # 4 key tiles per quarter

_BUILD_CACHE = {}


_DT = {"f32": dt.float32, "f32r": dt.float32r, "bf16": dt.bfloat16, "f16": dt.float16}


def _build(cfg):
    adt = _DT[cfg[0]]   # attention path: qkv/scores/PV operands
    pdt = _DT[cfg[1]]   # proj/fc/mlp path operands
    nc = bacc.Bacc("TRN2", target_bir_lowering=False, debug=False)

    hT = nc.dram_tensor("hT", [D, S], dt.float32, kind="ExternalInput")
    maskband = nc.dram_tensor("maskband", [Q, Q], dt.float32, kind="ExternalInput")
    maskb = nc.dram_tensor("maskb", [P, NQT * NKT], dt.float32, kind="ExternalInput")
    # weights arrive pre-tiled from the host in the exact consumption
    # order so every weight DMA is fully contiguous on HWDGE
    w_q = nc.dram_tensor("w_q", [DC, P, DC, P], adt, kind="ExternalInput")
    w_k = nc.dram_tensor("w_k", [DC, P, DC, P], adt, kind="ExternalInput")
    w_v = nc.dram_tensor("w_v", [2, P, DC, 512], adt, kind="ExternalInput")
    w_projr = nc.dram_tensor("w_projr", [DC, HD, H, P], pdt, kind="ExternalInput")
    w_fcr = nc.dram_tensor("w_fcr", [IC, P, DC, P], pdt, kind="ExternalInput")
    w_mlpr = nc.dram_tensor("w_mlpr", [2, DC, P, IC // 2, P], pdt, kind="ExternalInput")
    bq = nc.dram_tensor("bq", [P, DC], dt.float32, kind="ExternalInput")
    bk = nc.dram_tensor("bk", [P, DC], dt.float32, kind="ExternalInput")
    bv = nc.dram_tensor("bv", [HD, H], dt.float32, kind="ExternalInput")
    bproj = nc.dram_tensor("bproj", [P, DC], dt.float32, kind="ExternalInput")
    bfc = nc.dram_tensor("bfc", [P, IC], dt.float32, kind="ExternalInput")
    bmlp = nc.dram_tensor("bmlp", [P, DC], dt.float32, kind="ExternalInput")
    g1 = nc.dram_tensor("g1", [P, DC], dt.float32, kind="ExternalInput")
    be1 = nc.dram_tensor("be1", [P, DC], dt.float32, kind="ExternalInput")
    g2 = nc.dram_tensor("g2", [P, DC], dt.float32, kind="ExternalInput")
    be2 = nc.dram_tensor("be2", [P, DC], dt.float32, kind="ExternalInput")
    outT = nc.dram_tensor("outT", [D, Q], dt.float32, kind="ExternalOutput")

    hT_r = hT.rearrange("(c p) n -> p c n", p=P)
    maskband_r = maskband.rearrange("(k p) n -> p k n", p=P)

    with tile.TileContext(nc) as tc:
        with (
            tc.tile_pool(name="const", bufs=1) as const,
            tc.tile_pool(name="stats", bufs=1) as stats,
            tc.tile_pool(name="tmp", bufs=2) as tmp,
            tc.tile_pool(name="hstream", bufs=6) as hstream,
            tc.tile_pool(name="persist", bufs=1) as persist,
            tc.tile_pool(name="ps", bufs=4, space="PSUM") as ps,
            tc.tile_pool(name="lnps", bufs=2, space="PSUM") as lnps,
        ):
            ones_col = const.tile([P, 1], dt.float32)
            nc.vector.memset(ones_col[:], 1.0)
            ones_row = const.tile([1, P], dt.float32)
            nc.vector.memset(ones_row[:], 1.0)
            ones65 = const.tile([65, HD], dt.float32)
            nc.vector.memset(ones65[:], 1.0)
            eps_t = const.tile([1, 1], dt.float32)
            nc.vector.memset(eps_t[:], EPS)

            def load_pvec(t):
                s = const.tile(list(t.shape), dt.float32, tag=t.name)
                nc.sync.dma_start(s[:], t[:])
                return s

            maskb_s = load_pvec(maskb)
            bq_s, bk_s, bv_s = load_pvec(bq), load_pvec(bk), load_pvec(bv)
            bproj_s, bfc_s, bmlp_s = load_pvec(bproj), load_pvec(bfc), load_pvec(bmlp)
            g1_s, be1_s = load_pvec(g1), load_pvec(be1)
            g2_s, be2_s = load_pvec(g2), load_pvec(be2)

            # LN in transposed layout. get_chunk(c, keep) returns a [P, Q]
            # fp32 AP for chunk c (called for stats pass and apply pass).
            # Column stats via ones-matmuls; mean/rstd broadcast across
            # partitions via PE outer products.
            def layernorm_T(get_chunk, g_s, be_s, odt, out_pool, tag,
                            stats_on_pe=False):
                pss = lnps.tile([1, Q], dt.float32, tag="lnps")
                psq = lnps.tile([1, Q], dt.float32, tag="lnps")
                if stats_on_pe:
                    for c in range(DC):
                        xc = get_chunk(c)
                        sq = tmp.tile([P, Q], dt.float32, tag="sq")
                        nc.vector.tensor_tensor(sq[:], xc, xc, Alu.mult)
                        nc.tensor.matmul(pss[:], ones_col[:], xc,
                                         start=(c == 0), stop=(c == DC - 1))
                        nc.tensor.matmul(psq[:], ones_col[:], sq[:],
                                         start=(c == 0), stop=(c == DC - 1))
                else:
                    acc = tmp.tile([P, Q], dt.float32, tag="lnacc")
                    accq = tmp.tile([P, Q], dt.float32, tag="lnaccq")
                    for c in range(DC):
                        xc = get_chunk(c)
                        if c == 0:
                            nc.vector.tensor_copy(acc[:], xc)
                            nc.vector.tensor_tensor(accq[:], xc, xc, Alu.mult)
                        else:
                            nc.vector.tensor_tensor(acc[:], acc[:], xc, Alu.add)
                            sq = tmp.tile([P, Q], dt.float32, tag="sq")
                            nc.vector.tensor_tensor(sq[:], xc, xc, Alu.mult)
                            nc.vector.tensor_tensor(accq[:], accq[:], sq[:],
                                                    Alu.add)
                    nc.tensor.matmul(pss[:], ones_col[:], acc[:],
                                     start=True, stop=True)
                    nc.tensor.matmul(psq[:], ones_col[:], accq[:],
                                     start=True, stop=True)
                mean = stats.tile([1, Q], dt.float32, tag="mean")
                nc.vector.tensor_scalar_mul(mean[:], pss[:], 1.0 / D)
                msq = stats.tile([1, Q], dt.float32, tag="msq")
                nc.vector.tensor_tensor(msq[:], mean[:], mean[:], Alu.mult)
                var = stats.tile([1, Q], dt.float32, tag="var")
                nc.vector.scalar_tensor_tensor(
                    var[:], psq[:], 1.0 / D, msq[:], Alu.mult, Alu.subtract
                )
                nc.scalar.activation(msq[:], var[:], F.Sqrt, bias=eps_t[:])
                nc.vector.reciprocal(msq[:], msq[:])  # msq now holds rstd
                mb = lnps.tile([P, Q], dt.float32, tag="lnps")
                rb = lnps.tile([P, Q], dt.float32, tag="lnps")
                nc.tensor.matmul(mb[:], ones_row[:], mean[:], start=True, stop=True)
                nc.tensor.matmul(rb[:], ones_row[:], msq[:], start=True, stop=True)
                out = out_pool.tile([P, DC, Q], odt, tag=tag)
                for c in range(DC):
                    xc = get_chunk(c)
                    t1 = tmp.tile([P, Q], dt.float32, tag="lnt1")
                    nc.vector.tensor_tensor(t1[:], xc, mb[:], Alu.subtract)
                    nc.vector.scalar_tensor_tensor(
                        out[:, c, :], t1[:], g_s[:, c : c + 1], rb[:],
                        Alu.mult, Alu.mult,
                    )
                    nc.vector.tensor_scalar_add(
                        out[:, c, :], out[:, c, :], be_s[:, c : c + 1]
                    )
                return out

            def resident_chunks(x_sb):
                return lambda c: x_sb[:, c, :]

            h2 = persist.tile([P, DC, Q], dt.float32, tag="h2")

            with tc.tile_pool(name="attnsc", bufs=1) as attnsc:
                qt = attnsc.tile([P, DC, Q], adt, tag="qt")
                attn_acc = attnsc.tile([65, H, Q], dt.float32, tag="attn_acc")
                v_sb = attnsc.tile([P, NKT, H * 65], adt, tag="v")
                vview = v_sb[:].rearrange("p k (h x) -> p k h x", x=65)
                nc.vector.tensor_copy(
                    vview[:, :, :, 64:65],
                    ones_col[:].to_broadcast([P, NKT, H, 1]),
                )
                hq_sb = attnsc.tile([P, DC, Q], dt.float32, tag="hq")
                for c in range(DC):
                    nc.sync.dma_start(hq_sb[:, c, :], hT_r[:, c, 0:Q])

                with (
                    tc.tile_pool(name="quarter", bufs=1) as quarter,
                    tc.tile_pool(name="wkv", bufs=4) as wkv,
                    tc.tile_pool(name="wvp", bufs=2) as wvp,
                    tc.tile_pool(name="expp", bufs=6) as expp,
                    tc.tile_pool(name="pvps", bufs=2, space="PSUM") as pvps,
                ):
                    for q in range(NQT):
                        qsl = slice(q * KQ, (q + 1) * KQ)
                        if q == 0:
                            get_chunk = resident_chunks(hq_sb)
                        else:
                            def get_chunk(c, qsl=qsl):
                                hc = hstream.tile([P, Q], dt.float32, tag="hhc")
                                nc.sync.dma_start(hc[:], hT_r[:, c, qsl])
                                return hc[:]
                        xln = layernorm_T(get_chunk, g1_s, be1_s, adt,
                                          quarter, "xln", stats_on_pe=(q == 0))

                        if q == 0:
                            for p in range(DC):
                                wq_t = wkv.tile([P, DC, P], adt, tag="wq")
                                nc.sync.dma_start(wq_t[:], w_q[p])
                                psq_ = ps.tile([P, Q], dt.float32, tag="mm")
                                for c in range(DC):
                                    nc.tensor.matmul(
                                        psq_[:], wq_t[:, c, :], xln[:, c, :],
                                        start=(c == 0), stop=(c == DC - 1),
                                    )
                                nc.scalar.activation(
                                    qt[:, p, :], psq_[:], F.Identity,
                                    bias=bq_s[:, p : p + 1],
                                )

                        kt_sb = quarter.tile([P, DC, KQ], adt, tag="kt")
                        for p in range(DC):
                            wk_t = wkv.tile([P, DC, P], adt, tag="wq")
                            nc.sync.dma_start(wk_t[:], w_k[p])
                            psk = ps.tile([P, Q], dt.float32, tag="mm")
                            for c in range(DC):
                                nc.tensor.matmul(
                                    psk[:], wk_t[:, c, :], xln[:, c, :],
                                    start=(c == 0), stop=(c == DC - 1),
                                )
                            nc.scalar.activation(
                                kt_sb[:, p, :], psk[:], F.Identity,
                                bias=bk_s[:, p : p + 1],
                            )

                        for vs in range(2):
                            wv_t = wvp.tile([P, DC, 512], adt, tag="wv")
                            nc.sync.dma_start(wv_t[:], w_v[vs])
                            for kt in range(NKT):
                                psv = ps.tile([P, Q], dt.float32, tag="mm")
                                for c in range(DC):
                                    nc.tensor.matmul(
                                        psv[:],
                                        xln[:, c, kt * P : (kt + 1) * P],
                                        wv_t[:, c, :],
                                        start=(c == 0), stop=(c == DC - 1),
                                    )
                                dst = v_sb[
                                    :, kt, vs * 8 * 65 : (vs + 1) * 8 * 65
                                ].rearrange("p (h x) -> p h x", x=65)[:, :, 0:64]
                                nc.scalar.activation(
                                    dst,
                                    psv[:].rearrange("p (h x) -> p h x", x=64),
                                    F.Copy,
                                )

                        if q == 0:
                            mask_q = quarter.tile([P, NKT, Q], dt.float32,
                                                  tag="mask")
                            nc.sync.dma_start(mask_q[:], maskband_r[:])

                        for h in range(H):
                            hp, hs = h // 2, (h % 2) * 64
                            pa = pvps.tile([65, Q], dt.float32, tag="pv")
                            for kt in range(NKT):
                                pss = ps.tile([P, Q], dt.float32, tag="mm")
                                nc.tensor.matmul(
                                    pss[:],
                                    kt_sb[hs : hs + 64, hp, kt * P : (kt + 1) * P],
                                    qt[hs : hs + 64, hp, :],
                                    start=True, stop=True,
                                )
                                if q == 0:
                                    nc.vector.tensor_tensor(
                                        pss[:], pss[:], mask_q[:, kt, :], Alu.add
                                    )
                                et = expp.tile([P, Q], adt, tag="exp")
                                nc.scalar.activation(
                                    et[:], pss[:], F.Exp, scale=0.125,
                                    bias=maskb_s[:, q * NKT + kt : q * NKT + kt + 1],
                                )
                                nc.tensor.matmul(
                                    pa[:], v_sb[:, kt, h * 65 : h * 65 + 65],
                                    et[:],
                                    start=(kt == 0), stop=(kt == NKT - 1),
                                )
                            if q == 0:
                                nc.scalar.activation(
                                    attn_acc[:, h, :], pa[:], F.Copy
                                )
                            else:
                                nc.vector.tensor_tensor(
                                    attn_acc[:, h, :], attn_acc[:, h, :],
                                    pa[:], Alu.add,
                                )

                # normalize per head -> attnT [64, H, Q], then proj as a
                # plain GEMM over the 16 head-chunks + residual -> h2.
                with tc.tile_pool(name="projsc", bufs=1) as projsc, \
                     tc.tile_pool(name="pstream", bufs=3) as pstream:
                    attnT = projsc.tile([HD, H, Q], pdt, tag="attnT")
                    for h in range(H):
                        nc.vector.reciprocal(
                            attn_acc[64:65, h, :], attn_acc[64:65, h, :]
                        )
                        bc = lnps.tile([P, Q], dt.float32, tag="lnps")
                        nc.tensor.matmul(
                            bc[0:64, :], ones65[64:65, :],
                            attn_acc[64:65, h, :], start=True, stop=True,
                        )
                        t1 = tmp.tile([HD, Q], dt.float32, tag="anorm")
                        nc.vector.tensor_tensor(
                            t1[:], attn_acc[0:64, h, :], bc[0:64, :], Alu.mult
                        )
                        nc.vector.tensor_scalar_add(
                            attnT[:, h, :], t1[:], bv_s[:, h : h + 1]
                        )
                    for mo in range(DC):
                        wp_t = pstream.tile([HD, H, P], pdt, tag="wp")
                        nc.sync.dma_start(wp_t[:], w_projr[mo])
                        psp = ps.tile([P, Q], dt.float32, tag="mm")
                        for c in range(H):
                            nc.tensor.matmul(
                                psp[:], wp_t[:, c, :], attnT[:, c, :],
                                start=(c == 0), stop=(c == H - 1),
                            )
                        nc.vector.scalar_tensor_tensor(
                            h2[:, mo, :], psp[:], bproj_s[:, mo : mo + 1],
                            hq_sb[:, mo, :], Alu.add, Alu.add,
                        )

            # ---- LN2 / fc+gelu / mlp + residual ----
            with (
                tc.tile_pool(name="mlpsc", bufs=1) as mlpsc,
                tc.tile_pool(name="wfcs", bufs=4) as wfcs,
                tc.tile_pool(name="wmlps", bufs=4) as wmlps,
            ):
                h2n = layernorm_T(resident_chunks(h2), g2_s, be2_s, pdt,
                                  mlpsc, "h2n", stats_on_pe=True)
                y2 = mlpsc.tile([P, DC, Q], dt.float32, tag="y2")
                g_half = mlpsc.tile([P, IC // 2, Q], pdt, tag="g")
                for ih in range(2):
                    for m in range(IC // 2):
                        mg = ih * (IC // 2) + m
                        wfc_t = wfcs.tile([P, DC, P], pdt, tag="wfc")
                        nc.sync.dma_start(wfc_t[:], w_fcr[mg])
                        psf = ps.tile([P, Q], dt.float32, tag="mm")
                        for c in range(DC):
                            nc.tensor.matmul(
                                psf[:], wfc_t[:, c, :], h2n[:, c, :],
                                start=(c == 0), stop=(c == DC - 1),
                            )
                        nc.scalar.activation(
                            g_half[:, m, :], psf[:], F.Gelu,
                            bias=bfc_s[:, mg : mg + 1],
                        )
                    for mo in range(DC):
                        wm_t = wmlps.tile([P, IC // 2, P], pdt, tag="wmlp")
                        nc.sync.dma_start(wm_t[:], w_mlpr[ih, mo])
                        psm = ps.tile([P, Q], dt.float32, tag="mm")
                        for c in range(IC // 2):
                            nc.tensor.matmul(
                                psm[:], wm_t[:, c, :], g_half[:, c, :],
                                start=(c == 0), stop=(c == IC // 2 - 1),
                            )
                        if ih == 0:
                            nc.scalar.activation(y2[:, mo, :], psm[:], F.Copy)
                        else:
                            ot = tmp.tile([P, Q], dt.float32, tag="outt")
                            nc.vector.tensor_tensor(
                                ot[:], y2[:, mo, :], psm[:], Alu.add
                            )
                            nc.vector.scalar_tensor_tensor(
                                ot[:], ot[:], bmlp_s[:, mo : mo + 1],
                                h2[:, mo, :], Alu.add, Alu.add,
                            )
                            nc.sync.dma_start(
                                outT.rearrange("(c p) n -> p c n", p=P)[:, mo, :],
                                ot[:],
                            )

    nc.compile()
    return nc


def _get_nc(cfg):
    if cfg not in _BUILD_CACHE:
        _BUILD_CACHE[cfg] = _build(cfg)
    return _BUILD_CACHE[cfg]


def _np_dt(name):
    if name == "bf16":
        import ml_dtypes
        return ml_dtypes.bfloat16
    if name == "f16":
        return np.float16
    return np.float32


def _prep_in_maps(inputs, cfg):
    adt_np, pdt_np = _np_dt(cfg[0]), _np_dt(cfg[1])
    h = np.asarray(inputs["hidden_states"], dtype=np.float32)
    b_qkv = np.asarray(inputs["b_qkv"], np.float32)
    w_qkv = np.asarray(inputs["w_qkv"], np.float32)

    def chunk_w(w, p=P):  # [Din, N] -> [p, Din//p, N]
        return np.ascontiguousarray(w.reshape(-1, p, w.shape[1]).transpose(1, 0, 2))

    def pvec(v, p=P):  # [n*p] -> [p, n]
        return np.ascontiguousarray(v.reshape(-1, p).T)

    def mslice(a, nsl):  # [p, c, n] -> [n//nsl, p, c, nsl]
        p, c, n = a.shape
        return np.ascontiguousarray(
            a.reshape(p, c, n // nsl, nsl).transpose(2, 0, 1, 3)
        )

    wq = mslice(chunk_w(w_qkv[:, 0:D]), P)
    wk = mslice(chunk_w(w_qkv[:, D : 2 * D]), P)
    wv = mslice(chunk_w(w_qkv[:, 2 * D : 3 * D]), 512)
    w_proj = np.asarray(inputs["w_proj"], np.float32)
    wp = np.ascontiguousarray(
        w_proj.reshape(H, HD, DC, P).transpose(2, 1, 0, 3)
    )
    wfc = mslice(chunk_w(np.asarray(inputs["w_fc"], np.float32)), P)
    wm = chunk_w(np.asarray(inputs["w_mlp"], np.float32))  # [128, 32, 1024]
    wmlp = np.ascontiguousarray(
        wm.reshape(P, 2, IC // 2, DC, P).transpose(1, 3, 0, 2, 4)
    )
    vis = np.arange(Q)[:, None] <= np.arange(Q)[None, :]  # key i visible to query u
    maskband = np.where(vis, np.float32(0.0), np.float32(-10000.0))

    shared = {
        "w_q": wq.astype(adt_np), "w_k": wk.astype(adt_np),
        "w_v": wv.astype(adt_np), "w_projr": wp.astype(pdt_np),
        "w_fcr": wfc.astype(pdt_np), "w_mlpr": wmlp.astype(pdt_np),
        "maskband": np.ascontiguousarray(maskband.astype(np.float32)),
        "bq": pvec(b_qkv[0:D]),
        "bk": pvec(b_qkv[D : 2 * D]),
        "bv": pvec(b_qkv[2 * D : 3 * D], p=HD),
        "bproj": pvec(np.asarray(inputs["b_proj"], np.float32)),
        "bfc": pvec(np.asarray(inputs["b_fc"], np.float32)),
        "bmlp": pvec(np.asarray(inputs["b_mlp"], np.float32)),
        "g1": pvec(np.asarray(inputs["g1"], np.float32)),
        "be1": pvec(np.asarray(inputs["be1"], np.float32)),
        "g2": pvec(np.asarray(inputs["g2"], np.float32)),
        "be2": pvec(np.asarray(inputs["be2"], np.float32)),
    }
    in_maps = []
    for core in range(8):
        b, j = core // 4, core % 4
        perm = (np.arange(S) + j * Q) % S  # own rows become keys 0..511
        hrot = h[b, perm]
        # per-key exp bias: -10000/8 for keys strictly after the own
        # block (never visible); 0 otherwise (quarter 0 is handled by
        # the triangular band mask).
        masked = perm >= (j + 1) * Q
        mb = np.where(masked, np.float32(-1250.0), np.float32(0.0))
        maskb = np.ascontiguousarray(mb.reshape(NQT * NKT, P).T)
        in_maps.append(
            dict(
                shared,
                hT=np.ascontiguousarray(hrot.T),
                maskb=maskb.astype(np.float32),
            )
        )
    return in_maps


def _stitch(results):
    out = np.empty((2, S, D), dtype=np.float32)
    for core in range(8):
        b, j = core // 4, core % 4
        out[b, j * Q : (j + 1) * Q] = results[core]["outT"].T
    return out


def run(inputs, cfg=("bf16", "bf16"), trace=False, trace_cores=None):
    nc = _get_nc(cfg)
    in_maps = _prep_in_maps(inputs, cfg)
    res = bass_utils.run_bass_kernel_spmd(
        nc, in_maps, core_ids=list(range(8)), trace=trace, trace_cores=trace_cores
    )
    return _stitch(res.results), res


def kernel(**inputs) -> np.ndarray:
    out, _ = run(inputs, cfg=("bf16", "bf16"))
    return out



# revision 24
# speedup vs baseline: 1.1125x; 1.1125x over previous
"""GPT2 block kernel for 8 TRN2 NeuronCores (Bass/Tile, SPMD).

Sharding: the 4096 rows (batch*seq) are split 8 ways -> 512 rows/core
(4 cores per batch element). Core (b, a) owns query blocks {4j + a}
(128 rows each, one per key-quarter j) of batch b. Each core
redundantly computes K,V for its batch, but scores/PV/exp only for
(quarter q) x (own query tiles j >= q) -- 62.5% of the full rectangle.
Quarters are processed in DESCENDING order so query tile j (produced
from quarter j's LayerNorm output) exists before quarters q < j consume
it. Zero collectives.

Per-core key permutation (host side): within each quarter, the core's
own 128 rows are moved to the last 128 key positions, so the Q-proj
input is always xln[:, :, 384:512] -- uniform addresses across cores;
the causal mask becomes a per-core data tensor (multiplicative 0/1 on
the exp'd scores, applied by the otherwise-idle GpSimd engine).

LN gamma/beta are folded into the consuming weights host-side, exp is
batched into [128, 2, N] activations, reciprocals use the fast approx
DVE op, and proj packs head pairs to a full K=128 contraction.
"""

import numpy as np
import sys

sys.path.insert(0, "/opt/trn_rl_repo")

import concourse.bacc as bacc
import concourse.mybir as mybir
import concourse.tile as tile
from concourse import bass_utils

dt = mybir.dt
F = mybir.ActivationFunctionType
Alu = mybir.AluOpType

D = 1024
S = 2048
Q = 512        # own rows per core
H = 16
HD = 64
INNER = 4096
P = 128
DC = D // P    # 8
IC = INNER // P  # 32
EPS = 1e-5
NQT = 4        # key quarters
KQ = S // NQT  # 512 keys per quarter
NKT = KQ // P  # 4 key tiles of 128 per quarter

_BUILD_CACHE = {}

_DT = {"f32": dt.float32, "f32r": dt.float32r, "bf16": dt.bfloat16, "f16": dt.float16}


def _build(cfg, dbg=False):
    adt = _DT[cfg[0]]   # attention path: qkv/scores/PV operands
    pdt = _DT[cfg[1]]   # proj/fc/mlp path operands
    nc = bacc.Bacc("TRN2", target_bir_lowering=False, debug=False)
    if dbg:
        dbg_xln = nc.dram_tensor("dbg_xln", [NQT, P, DC, KQ], adt,
                                 kind="ExternalOutput")
        dbg_qt = nc.dram_tensor("dbg_qt", [P, DC, Q], adt,
                                kind="ExternalOutput")
        dbg_kt = nc.dram_tensor("dbg_kt", [NQT, P, DC, KQ], adt,
                                kind="ExternalOutput")
        dbg_acc = nc.dram_tensor("dbg_acc", [65, H, Q], dt.float32,
                                 kind="ExternalOutput")
        dbg_et = nc.dram_tensor("dbg_et", [NQT, P, NKT, KQ], adt,
                                kind="ExternalOutput")
        dbg_at2 = nc.dram_tensor("dbg_at2", [P, DC, Q], pdt,
                                 kind="ExternalOutput")
        dbg_h2 = nc.dram_tensor("dbg_h2", [P, DC, Q], dt.float32,
                                kind="ExternalOutput")
        dbg_h2n = nc.dram_tensor("dbg_h2n", [P, DC, Q], pdt,
                                 kind="ExternalOutput")
        dbg_g = nc.dram_tensor("dbg_g", [P, IC // 2, Q], pdt,
                               kind="ExternalOutput")

    hT = nc.dram_tensor("hT", [D, S], dt.float32, kind="ExternalInput")
    dmask = nc.dram_tensor("dmask", [P, NKT, P], adt, kind="ExternalInput")
    # weights arrive pre-tiled from the host in consumption order
    w_q = nc.dram_tensor("w_q", [DC, P, DC, P], adt, kind="ExternalInput")
    w_k = nc.dram_tensor("w_k", [DC, P, DC, P], adt, kind="ExternalInput")
    w_v = nc.dram_tensor("w_v", [2, P, DC, 512], adt, kind="ExternalInput")
    w_projr = nc.dram_tensor("w_projr", [DC, P, DC, P], pdt, kind="ExternalInput")
    w_fcr = nc.dram_tensor("w_fcr", [IC, P, DC, P], pdt, kind="ExternalInput")
    w_mlpr = nc.dram_tensor("w_mlpr", [2, DC, P, IC // 2, P], pdt, kind="ExternalInput")
    bq = nc.dram_tensor("bq", [P, DC], dt.float32, kind="ExternalInput")
    bk = nc.dram_tensor("bk", [P, DC], dt.float32, kind="ExternalInput")
    bv = nc.dram_tensor("bv", [HD, H], dt.float32, kind="ExternalInput")
    bproj = nc.dram_tensor("bproj", [P, DC], dt.float32, kind="ExternalInput")
    bfc = nc.dram_tensor("bfc", [P, IC], dt.float32, kind="ExternalInput")
    bmlp = nc.dram_tensor("bmlp", [P, DC], dt.float32, kind="ExternalInput")
    outT = nc.dram_tensor("outT", [D, Q], dt.float32, kind="ExternalOutput")

    hT_r = hT.rearrange("(c p) n -> p c n", p=P)

    with tile.TileContext(nc) as tc:
        with (
            tc.tile_pool(name="const", bufs=1) as const,
            tc.tile_pool(name="rows", bufs=2) as rows,
            tc.tile_pool(name="tmp", bufs=3) as tmp,
            tc.tile_pool(name="rowtmp", bufs=2) as rowtmp,
            tc.tile_pool(name="persist", bufs=1) as persist,
        ):
            ones_col = const.tile([P, 1], dt.float32)
            nc.vector.memset(ones_col[:], 1.0)
            ones_row = const.tile([1, P], dt.float32)
            nc.vector.memset(ones_row[:], 1.0)
            eps_t = const.tile([1, 1], dt.float32)
            nc.vector.memset(eps_t[:], EPS)

            def load_pvec(t):
                s = const.tile(list(t.shape), dt.float32, tag=t.name)
                nc.sync.dma_start(s[:], t[:])
                return s

            bq_s, bk_s, bv_s = load_pvec(bq), load_pvec(bk), load_pvec(bv)
            bproj_s, bfc_s, bmlp_s = load_pvec(bproj), load_pvec(bfc), load_pvec(bmlp)

            h2 = persist.tile([P, DC, Q], dt.float32, tag="h2")
            hq_sb = persist.tile([P, DC, Q], dt.float32, tag="hq")

            # LN stats for a [P, DC, KQ] fp32 block resident in SBUF.
            # sq on GpSimd, partition-sums via PE ones-matmuls, row math on
            # DVE, sqrt on scalar, reciprocal via fast DVE approx.
            def ln_stats(get_chunk, lnps, tag):
                pss = lnps.tile([1, KQ], dt.float32, tag="lnrow")
                psq = lnps.tile([1, KQ], dt.float32, tag="lnrow")
                for c in range(DC):
                    xc = get_chunk(c)
                    sq = tmp.tile([P, KQ], dt.float32, tag="sq")
                    nc.gpsimd.tensor_tensor(sq[:], xc, xc, Alu.mult)
                    nc.tensor.matmul(pss[:], ones_col[:], xc,
                                     start=(c == 0), stop=(c == DC - 1))
                    nc.tensor.matmul(psq[:], ones_col[:], sq[:],
                                     start=(c == 0), stop=(c == DC - 1))
                mean = rows.tile([1, KQ], dt.float32, tag="mean")
                nc.vector.tensor_scalar_mul(mean[:], pss[:], 1.0 / D)
                msq = rowtmp.tile([1, KQ], dt.float32, tag="lnrow")
                nc.vector.tensor_tensor(msq[:], mean[:], mean[:], Alu.mult)
                var = rowtmp.tile([1, KQ], dt.float32, tag="lnrow")
                nc.vector.scalar_tensor_tensor(
                    var[:], psq[:], 1.0 / D, msq[:], Alu.mult, Alu.subtract
                )
                std = rowtmp.tile([1, KQ], dt.float32, tag="lnrow")
                nc.scalar.activation(std[:], var[:], F.Sqrt, bias=eps_t[:])
                rstd = rows.tile([1, KQ], dt.float32, tag="rstd")
                nc.vector.reciprocal_approx_fast(rstd[:], std[:])
                return mean, rstd

            # apply pass: xln = (x - mean) * rstd (gamma/beta folded into
            # the consuming weights host-side), written as `odt`.
            def ln_apply_start(mean, rstd, bbpool, bbtag):
                mb = bbpool.tile([P, KQ], dt.float32, tag=bbtag)
                rb = bbpool.tile([P, KQ], dt.float32, tag=bbtag)
                nc.tensor.matmul(mb[:], ones_row[:], mean[:], start=True, stop=True)
                nc.tensor.matmul(rb[:], ones_row[:], rstd[:], start=True, stop=True)
                return mb, rb

            def ln_apply_chunk(get_chunk, mb, rb, out, c):
                xc = get_chunk(c)
                t1 = tmp.tile([P, KQ], dt.float32, tag="lnt1")
                nc.vector.tensor_tensor(t1[:], xc, mb[:], Alu.subtract)
                nc.vector.tensor_tensor(out[:, c, :], t1[:], rb[:], Alu.mult)

            def ln_apply(get_chunk, mean, rstd, out, bbpool, bbtag):
                mb, rb = ln_apply_start(mean, rstd, bbpool, bbtag)
                for c in range(DC):
                    ln_apply_chunk(get_chunk, mb, rb, out, c)

            with (
                tc.tile_pool(name="attnsc", bufs=1) as attnsc,
                tc.tile_pool(name="hqp", bufs=2) as hqp,
                tc.tile_pool(name="xlnp", bufs=1) as xlnp,
                tc.tile_pool(name="wkv", bufs=2) as wkv,
                tc.tile_pool(name="wvp", bufs=1) as wvp,
                tc.tile_pool(name="etp", bufs=2) as etp,
                tc.tile_pool(name="ps", bufs=2, space="PSUM") as ps,
                tc.tile_pool(name="pvps", bufs=2, space="PSUM") as pvps,
                tc.tile_pool(name="lnps", bufs=2, space="PSUM") as lnps,
            ):
                qt = attnsc.tile([P, DC, Q], adt, tag="qt")
                attn_acc = attnsc.tile([65, H, Q], dt.float32, tag="attn_acc")
                kt_sb = attnsc.tile([P, DC, KQ], adt, tag="kt")
                v_sb = attnsc.tile([P, NKT, H * 65], adt, tag="v")
                vview = v_sb[:].rearrange("p k (h x) -> p k h x", x=65)
                nc.vector.tensor_copy(
                    vview[:, :, :, 64:65],
                    ones_col[:].to_broadcast([P, NKT, H, 1]),
                )
                dmask_s = const.tile([P, NKT, P], adt, tag="dmask")
                nc.sync.dma_start(dmask_s[:], dmask[:])

                # own rows (last 128 of each quarter) for the residual path
                for c in range(DC):
                    nc.sync.dma_start(
                        hq_sb[:, c, :],
                        hT_r[:, c, :].rearrange(
                            "p (j n) -> p j n", n=KQ
                        )[:, :, 384:512],
                    )

                hquart = {}

                def load_quarter(q):
                    t = hqp.tile([P, DC, KQ], dt.float32, tag="hquart")
                    for c in range(DC):
                        nc.sync.dma_start(
                            t[:, c, :], hT_r[:, c, q * KQ:(q + 1) * KQ]
                        )
                    hquart[q] = t

                def hchunk(q):
                    return lambda c: hquart[q][:, c, :]

                load_quarter(3)
                stats = {3: ln_stats(hchunk(3), lnps, "3")}

                xln_t = {}

                def make_xln(q):
                    xln = xlnp.tile([P, DC, KQ], adt, tag="xln")
                    mean, rstd = stats.pop(q)
                    ln_apply(hchunk(q), mean, rstd, xln, lnps, "lnrow")
                    xln_t[q] = xln

                make_xln(3)
                for q in range(NQT - 1, -1, -1):
                    xln = xln_t.pop(q)

                    # K projection for this quarter's 512 keys
                    for p in range(DC):
                        wk_t = wkv.tile([P, DC, P], adt, tag="wkq")
                        nc.sync.dma_start(wk_t[:], w_k[p])
                        psk = ps.tile([P, KQ], dt.float32, tag="mm")
                        for c in range(DC):
                            nc.tensor.matmul(
                                psk[:], wk_t[:, c, :], xln[:, c, :],
                                start=(c == 0), stop=(c == DC - 1),
                            )
                        nc.vector.tensor_scalar_add(
                            kt_sb[:, p, :], psk[:], bk_s[:, p:p + 1]
                        )

                    # V projection (keys on partitions)
                    for vs in range(2):
                        wv_t = wvp.tile([P, DC, 512], adt, tag="wv")
                        nc.sync.dma_start(wv_t[:], w_v[vs])
                        for kt in range(NKT):
                            psv = ps.tile([P, KQ], dt.float32, tag="mm")
                            for c in range(DC):
                                nc.tensor.matmul(
                                    psv[:],
                                    xln[:, c, kt * P:(kt + 1) * P],
                                    wv_t[:, c, :],
                                    start=(c == 0), stop=(c == DC - 1),
                                )
                            dst = v_sb[
                                :, kt, vs * 8 * 65:(vs + 1) * 8 * 65
                            ].rearrange("p (h x) -> p h x", x=65)[:, :, 0:64]
                            nc.vector.tensor_copy(
                                dst, psv[:].rearrange("p (h x) -> p h x", x=64)
                            )

                    # Q projection for own query tile j=q (last 128 keys of
                    # this quarter are the core's own rows)
                    for p in range(DC):
                        wq_t = wkv.tile([P, DC, P], adt, tag="wkq")
                        nc.sync.dma_start(wq_t[:], w_q[p])
                        psq_ = ps.tile([P, KQ], dt.float32, tag="mm")
                        for c in range(DC):
                            nc.tensor.matmul(
                                psq_[:, 0:P], wq_t[:, c, :], xln[:, c, 384:512],
                                start=(c == 0), stop=(c == DC - 1),
                            )
                        nc.vector.tensor_scalar_add(
                            qt[:, p, q * P:(q + 1) * P], psq_[:, 0:P],
                            bq_s[:, p:p + 1],
                        )

                    # LN stats for the next (lower) quarter; the 8 apply
                    # chunks are interleaved into the head loop below so the
                    # DVE queue stays smooth
                    nxt = None
                    if q > 0:
                        load_quarter(q - 1)
                        stats[q - 1] = ln_stats(hchunk(q - 1), lnps, str(q - 1))
                        xn = xlnp.tile([P, DC, KQ], adt, tag="xln")
                        mean_n, rstd_n = stats.pop(q - 1)
                        mb_n, rb_n = ln_apply_start(mean_n, rstd_n, lnps, "lnrow")
                        xln_t[q - 1] = xn
                        nxt = (mb_n, rb_n, xn)

                    if dbg:
                        nc.sync.dma_start(dbg_xln[q], xln[:])
                        nc.sync.dma_start(dbg_kt[q], kt_sb[:])

                    # head loop: scores/exp/PV for query tiles j >= q
                    NQ = (NQT - q) * P
                    qsl = slice(q * P, Q)
                    for h in range(H):
                        hp, hs = h // 2, (h % 2) * 64
                        et = etp.tile([P, NKT, KQ], adt, tag="et")
                        pa = pvps.tile([65, KQ], dt.float32, tag="pv")
                        for kt in range(NKT):
                            psc = ps.tile([P, KQ], dt.float32, tag="mm")
                            nc.tensor.matmul(
                                psc[:, 0:NQ],
                                kt_sb[hs:hs + 64, hp, kt * P:(kt + 1) * P],
                                qt[hs:hs + 64, hp, qsl],
                                start=True, stop=True,
                            )
                            nc.scalar.activation(
                                et[:, kt, 0:NQ], psc[:, 0:NQ],
                                F.Exp, scale=0.125,
                            )
                            if kt % 2 == 1:
                                # multiplicative causal mask on the diagonal
                                # query tile (first 128 columns of the window)
                                nc.vector.tensor_tensor(
                                    et[:, kt - 1:kt + 1, 0:P],
                                    et[:, kt - 1:kt + 1, 0:P],
                                    dmask_s[:, kt - 1:kt + 1, :], Alu.mult,
                                )
                        for kt in range(NKT):
                            nc.tensor.matmul(
                                pa[:, qsl], v_sb[:, kt, h * 65:h * 65 + 65],
                                et[:, kt, 0:NQ],
                                start=(kt == 0), stop=(kt == NKT - 1),
                            )
                        nc.scalar.activation(
                            attn_acc[:, h, q * P:(q + 1) * P],
                            pa[:, q * P:(q + 1) * P], F.Copy,
                        )
                        if nxt is not None and h % 2 == 1:
                            ln_apply_chunk(hchunk(q - 1), nxt[0], nxt[1],
                                           nxt[2], h // 2)
                        if q < NQT - 1:
                            nc.vector.tensor_tensor(
                                attn_acc[:, h, (q + 1) * P:],
                                attn_acc[:, h, (q + 1) * P:],
                                pa[:, (q + 1) * P:], Alu.add,
                            )
                        if dbg and h == 0:
                            nc.sync.dma_start(dbg_et[q], et[:])

                if dbg:
                    nc.sync.dma_start(dbg_qt[:], qt[:])
                    nc.sync.dma_start(dbg_acc[:], attn_acc[:])

                # normalize per head into head-pair-packed attnT2
                # [128 = heads (h%8==c, rows 0-63) & (c+8, rows 64-127), 8, Q]
                with tc.tile_pool(name="projsc", bufs=1) as projsc, \
                     tc.tile_pool(name="pstream", bufs=2) as pstream:
                    attnT2 = projsc.tile([P, DC, Q], pdt, tag="attnT2")
                    for h in range(H):
                        # copy to base partition 0: the custom DVE op does
                        # not honor a nonzero base_partition on its input
                        srow = rowtmp.tile([1, Q], dt.float32, tag="srow")
                        nc.vector.tensor_copy(srow[:], attn_acc[64:65, h, :])
                        rrow = rowtmp.tile([1, Q], dt.float32, tag="rrow")
                        nc.vector.reciprocal_approx_fast(rrow[:], srow[:])
                        bc = pvps.tile([P, Q], dt.float32, tag="pv")
                        nc.tensor.matmul(
                            bc[0:64, :], ones_row[0:1, 0:64], rrow[:],
                            start=True, stop=True,
                        )
                        t1 = tmp.tile([HD, Q], dt.float32, tag="anorm")
                        nc.vector.tensor_tensor(
                            t1[:], attn_acc[0:64, h, :], bc[0:64, :], Alu.mult
                        )
                        off = 64 * (h // 8)
                        nc.vector.tensor_scalar_add(
                            attnT2[off:off + 64, h % 8, :], t1[:],
                            bv_s[:, h:h + 1],
                        )
                    if dbg:
                        nc.sync.dma_start(dbg_at2[:], attnT2[:])
                    for mo in range(DC):
                        wp_t = pstream.tile([P, DC, P], pdt, tag="wp")
                        nc.sync.dma_start(wp_t[:], w_projr[mo])
                        psp = ps.tile([P, 2, KQ], dt.float32, tag="mm")
                        for c in range(DC):
                            nc.tensor.matmul(
                                psp[:, 0, :], wp_t[:, c, :], attnT2[:, c, :],
                                start=(c == 0), stop=(c == DC - 1),
                            )
                        nc.vector.scalar_tensor_tensor(
                            h2[:, mo, :], psp[:, 0, :], bproj_s[:, mo:mo + 1],
                            hq_sb[:, mo, :], Alu.add, Alu.add,
                        )

            # ---- LN2 / fc+gelu / mlp + residual ----
            with (
                tc.tile_pool(name="mlpsc", bufs=1) as mlpsc,
                tc.tile_pool(name="wfcs", bufs=2) as wfcs,
                tc.tile_pool(name="wmlps", bufs=2) as wmlps,
                tc.tile_pool(name="psfc", bufs=2, space="PSUM") as psfc,
                tc.tile_pool(name="psm", bufs=2, space="PSUM") as psm,
                tc.tile_pool(name="lnps2", bufs=2, space="PSUM") as lnps2,
            ):
                h2c = lambda c: h2[:, c, :]
                mean2, rstd2 = ln_stats(h2c, lnps2, "h2")
                h2n = mlpsc.tile([P, DC, Q], pdt, tag="h2n")
                ln_apply(h2c, mean2, rstd2, h2n, lnps2, "lnbb")
                if dbg:
                    nc.sync.dma_start(dbg_h2[:], h2[:])
                    nc.sync.dma_start(dbg_h2n[:], h2n[:])
                y2 = mlpsc.tile([P, DC, Q], dt.float32, tag="y2")
                g_half = mlpsc.tile([P, IC // 2, Q], pdt, tag="g")
                for ih in range(2):
                    for m in range(IC // 2):
                        mg = ih * (IC // 2) + m
                        wfc_t = wfcs.tile([P, DC, P], pdt, tag="wfc")
                        nc.sync.dma_start(wfc_t[:], w_fcr[mg])
                        psf = psfc.tile([P, Q], dt.float32, tag="fc")
                        for c in range(DC):
                            nc.tensor.matmul(
                                psf[:], wfc_t[:, c, :], h2n[:, c, :],
                                start=(c == 0), stop=(c == DC - 1),
                            )
                        nc.scalar.activation(
                            g_half[:, m, :], psf[:], F.Gelu,
                            bias=bfc_s[:, mg:mg + 1],
                        )
                    if dbg and ih == 0:
                        nc.sync.dma_start(dbg_g[:], g_half[:])
                    for mo in range(DC):
                        wm_t = wmlps.tile([P, IC // 2, P], pdt, tag="wmlp")
                        nc.sync.dma_start(wm_t[:], w_mlpr[ih, mo])
                        psm_ = psm.tile([P, Q], dt.float32, tag="mm2")
                        for c in range(IC // 2):
                            nc.tensor.matmul(
                                psm_[:], wm_t[:, c, :], g_half[:, c, :],
                                start=(c == 0), stop=(c == IC // 2 - 1),
                            )
                        if ih == 0:
                            nc.vector.tensor_copy(y2[:, mo, :], psm_[:])
                        else:
                            ot = tmp.tile([P, Q], dt.float32, tag="outt")
                            nc.vector.tensor_tensor(
                                ot[:], y2[:, mo, :], psm_[:], Alu.add
                            )
                            nc.vector.scalar_tensor_tensor(
                                ot[:], ot[:], bmlp_s[:, mo:mo + 1],
                                h2[:, mo, :], Alu.add, Alu.add,
                            )
                            nc.sync.dma_start(
                                outT.rearrange("(c p) n -> p c n", p=P)[:, mo, :],
                                ot[:],
                            )

    nc.compile()
    return nc


def _get_nc(cfg):
    if cfg not in _BUILD_CACHE:
        _BUILD_CACHE[cfg] = _build(cfg)
    return _BUILD_CACHE[cfg]


def _np_dt(name):
    if name == "bf16":
        import ml_dtypes
        return ml_dtypes.bfloat16
    if name == "f16":
        return np.float16
    return np.float32


def _perm_for_core(a):
    """Key order per quarter: non-own keys in natural order, own 128 last."""
    perm = []
    for q in range(NQT):
        base = q * KQ
        own = np.arange(base + 128 * a, base + 128 * a + P)
        others = np.setdiff1d(np.arange(base, base + KQ), own)
        perm.append(np.concatenate([others, own]))
    return np.concatenate(perm)


def _prep_in_maps(inputs, cfg):
    adt_np, pdt_np = _np_dt(cfg[0]), _np_dt(cfg[1])
    h = np.asarray(inputs["hidden_states"], dtype=np.float32)
    w_qkv = np.asarray(inputs["w_qkv"], np.float32)
    b_qkv = np.asarray(inputs["b_qkv"], np.float32)
    g1 = np.asarray(inputs["g1"], np.float32)
    be1 = np.asarray(inputs["be1"], np.float32)
    g2 = np.asarray(inputs["g2"], np.float32)
    be2 = np.asarray(inputs["be2"], np.float32)

    # fold LN1 gamma/beta into the qkv weights and biases
    w_qkv_f = w_qkv * g1[:, None]
    b_qkv_f = b_qkv + be1 @ w_qkv
    w_fc = np.asarray(inputs["w_fc"], np.float32)
    b_fc = np.asarray(inputs["b_fc"], np.float32)
    w_fc_f = w_fc * g2[:, None]
    b_fc_f = b_fc + be2 @ w_fc

    def chunk_w(w, p=P):  # [Din, N] -> [p, Din//p, N]
        return np.ascontiguousarray(w.reshape(-1, p, w.shape[1]).transpose(1, 0, 2))

    def pvec(v, p=P):  # [n*p] -> [p, n]
        return np.ascontiguousarray(v.reshape(-1, p).T)

    def mslice(a, nsl):  # [p, c, n] -> [n//nsl, p, c, nsl]
        p, c, n = a.shape
        return np.ascontiguousarray(
            a.reshape(p, c, n // nsl, nsl).transpose(2, 0, 1, 3)
        )

    wq = mslice(chunk_w(w_qkv_f[:, 0:D]), P)
    wk = mslice(chunk_w(w_qkv_f[:, D:2 * D]), P)
    wv = mslice(chunk_w(w_qkv_f[:, 2 * D:3 * D]), 512)

    # proj with head pairs (c, c+8) stacked on the 128 contraction rows
    w_proj = np.asarray(inputs["w_proj"], np.float32)
    w2 = w_proj.reshape(H, HD, D)
    wp_t = np.concatenate([w2[0:8], w2[8:16]], axis=1)  # [8, 128, 1024]
    wp = np.ascontiguousarray(
        wp_t.reshape(DC, P, DC, P).transpose(2, 1, 0, 3)
    )

    wfc = mslice(chunk_w(w_fc_f), P)
    wm = chunk_w(np.asarray(inputs["w_mlp"], np.float32))  # [128, 32, 1024]
    wmlp = np.ascontiguousarray(
        wm.reshape(P, 2, IC // 2, DC, P).transpose(1, 3, 0, 2, 4)
    )

    shared = {
        "w_q": wq.astype(adt_np), "w_k": wk.astype(adt_np),
        "w_v": wv.astype(adt_np), "w_projr": wp.astype(pdt_np),
        "w_fcr": wfc.astype(pdt_np), "w_mlpr": wmlp.astype(pdt_np),
        "bq": pvec(b_qkv_f[0:D]),
        "bk": pvec(b_qkv_f[D:2 * D]),
        "bv": pvec(b_qkv_f[2 * D:3 * D], p=HD),
        "bproj": pvec(np.asarray(inputs["b_proj"], np.float32)),
        "bfc": pvec(b_fc_f),
        "bmlp": pvec(np.asarray(inputs["b_mlp"], np.float32)),
    }
    in_maps = []
    for core in range(8):
        b, a = core // 4, core % 4
        perm = _perm_for_core(a)
        hrot = h[b, perm]
        # multiplicative 0/1 mask on exp'd scores for the diagonal query
        # tile: key position j=kt*128+p in the quarter vs query row r.
        pp = np.arange(P)
        dm = np.zeros((P, NKT, P), np.float32)
        for kt in range(NKT - 1):
            dm[:, kt, :] = ((kt * P + pp) < (P * a)).astype(np.float32)[:, None]
        dm[:, NKT - 1, :] = (pp[:, None] <= pp[None, :]).astype(np.float32)
        in_maps.append(
            dict(
                shared,
                hT=np.ascontiguousarray(hrot.T),
                dmask=dm.astype(adt_np),
            )
        )
    return in_maps


def _stitch(results):
    out = np.empty((2, S, D), dtype=np.float32)
    for core in range(8):
        b, a = core // 4, core % 4
        r = results[core]["outT"].T  # [512, D]: cols j*128+p -> row 512j+128a+p
        for j in range(NQT):
            out[b, j * KQ + P * a: j * KQ + P * a + P] = r[j * P:(j + 1) * P]
    return out


def run(inputs, cfg=("bf16", "bf16"), trace=False, trace_cores=None):
    nc = _get_nc(cfg)
    in_maps = _prep_in_maps(inputs, cfg)
    res = bass_utils.run_bass_kernel_spmd(
        nc, in_maps, core_ids=list(range(8)), trace=trace, trace_cores=trace_cores
    )
    return _stitch(res.results), res


def kernel(**inputs) -> np.ndarray:
    out, _ = run(inputs, cfg=("bf16", "bf16"))
    return out


# revision 39
# speedup vs baseline: 1.5044x; 1.3523x over previous
"""GPT2 block kernel for 8 TRN2 NeuronCores (Bass/Tile, SPMD).

Sharding: the 4096 rows (batch*seq) are split 8 ways -> 512 rows/core
(4 cores per batch element). Core (b, a) owns query blocks {4j + a}
(128 rows each, one per key-quarter j) of batch b. Each core
redundantly computes K,V for its batch, but scores/PV/exp only for
(quarter q) x (own query tiles j >= q) -- 62.5% of the full rectangle.
Quarters are processed in DESCENDING order so query tile j (produced
from quarter j's LayerNorm output) exists before quarters q < j consume
it. Zero collectives.

Per-core key permutation (host side): within each quarter, the core's
own 128 rows are moved to the last 128 key positions, so the Q-proj
input is always xln[:, :, 384:512] -- uniform addresses across cores;
the causal mask becomes a per-core data tensor (multiplicative 0/1 on
the exp'd scores, applied by the otherwise-idle GpSimd engine).

LN gamma/beta are folded into the consuming weights host-side, exp is
batched into [128, 2, N] activations, reciprocals use the fast approx
DVE op, and proj packs head pairs to a full K=128 contraction.
"""

import numpy as np
import sys

sys.path.insert(0, "/opt/trn_rl_repo")

import concourse.bacc as bacc
import concourse.mybir as mybir
import concourse.tile as tile
from concourse import bass_utils

dt = mybir.dt
F = mybir.ActivationFunctionType
Alu = mybir.AluOpType

D = 1024
S = 2048
Q = 512        # own rows per core
H = 16
HD = 64
INNER = 4096
P = 128
DC = D // P    # 8
IC = INNER // P  # 32
EPS = 1e-5
NQT = 4        # key quarters
KQ = S // NQT  # 512 keys per quarter
NKT = KQ // P  # 4 key tiles of 128 per quarter

_BUILD_CACHE = {}

_DT = {"f32": dt.float32, "f32r": dt.float32r, "bf16": dt.bfloat16, "f16": dt.float16}


def _build(cfg, dbg=False):
    adt = _DT[cfg[0]]   # attention path: qkv/scores/PV operands
    pdt = _DT[cfg[1]]   # proj/fc/mlp path operands
    nc = bacc.Bacc("TRN2", target_bir_lowering=False, debug=False)
    if dbg:
        dbg_xln = nc.dram_tensor("dbg_xln", [NQT, P, DC, KQ], adt,
                                 kind="ExternalOutput")
        dbg_qt = nc.dram_tensor("dbg_qt", [P, DC, Q], adt,
                                kind="ExternalOutput")
        dbg_kt = nc.dram_tensor("dbg_kt", [NQT, P, DC, KQ], adt,
                                kind="ExternalOutput")
        dbg_acc = nc.dram_tensor("dbg_acc", [65, H, Q], dt.float32,
                                 kind="ExternalOutput")
        dbg_et = nc.dram_tensor("dbg_et", [NQT, P, NKT, KQ], adt,
                                kind="ExternalOutput")
        dbg_at2 = nc.dram_tensor("dbg_at2", [P, DC, Q], pdt,
                                 kind="ExternalOutput")
        dbg_h2 = nc.dram_tensor("dbg_h2", [P, DC, Q], dt.float32,
                                kind="ExternalOutput")
        dbg_h2n = nc.dram_tensor("dbg_h2n", [P, DC, Q], pdt,
                                 kind="ExternalOutput")
        dbg_g = nc.dram_tensor("dbg_g", [P, IC // 2, Q], pdt,
                               kind="ExternalOutput")

    hT = nc.dram_tensor("hT", [D, S], dt.float32, kind="ExternalInput")
    dmask = nc.dram_tensor("dmask", [P, NKT, P], adt, kind="ExternalInput")
    # weights arrive pre-tiled from the host in consumption order
    w_q = nc.dram_tensor("w_q", [DC, P, DC, P], adt, kind="ExternalInput")
    w_k = nc.dram_tensor("w_k", [DC, P, DC, P], adt, kind="ExternalInput")
    w_v = nc.dram_tensor("w_v", [2, P, DC, 512], adt, kind="ExternalInput")
    w_projr = nc.dram_tensor("w_projr", [DC, P, DC, P], pdt, kind="ExternalInput")
    w_fcr = nc.dram_tensor("w_fcr", [IC, P, DC, P], pdt, kind="ExternalInput")
    w_mlpr = nc.dram_tensor("w_mlpr", [2, DC, P, IC // 2, P], pdt, kind="ExternalInput")
    bq = nc.dram_tensor("bq", [P, DC], dt.float32, kind="ExternalInput")
    bk = nc.dram_tensor("bk", [P, DC], dt.float32, kind="ExternalInput")
    bv = nc.dram_tensor("bv", [HD, H], dt.float32, kind="ExternalInput")
    bproj = nc.dram_tensor("bproj", [P, DC], dt.float32, kind="ExternalInput")
    bfc = nc.dram_tensor("bfc", [P, IC], dt.float32, kind="ExternalInput")
    bmlp = nc.dram_tensor("bmlp", [P, DC], dt.float32, kind="ExternalInput")
    outT = nc.dram_tensor("outT", [D, Q], dt.float32, kind="ExternalOutput")

    hT_r = hT.rearrange("(c p) n -> p c n", p=P)

    with tile.TileContext(nc) as tc:
        with (
            tc.tile_pool(name="const", bufs=1) as const,
            tc.tile_pool(name="rows", bufs=2) as rows,
            tc.tile_pool(name="tmp", bufs=2) as tmp,
            tc.tile_pool(name="rowtmp", bufs=2) as rowtmp,
            tc.tile_pool(name="persist", bufs=1) as persist,
        ):
            ones_col = const.tile([P, 1], dt.float32)
            nc.vector.memset(ones_col[:], 1.0)
            ones_row = const.tile([1, P], dt.float32)
            nc.vector.memset(ones_row[:], 1.0)
            eps_t = const.tile([1, 1], dt.float32)
            nc.vector.memset(eps_t[:], EPS)

            def load_pvec(t):
                s = const.tile(list(t.shape), dt.float32, tag=t.name)
                nc.sync.dma_start(s[:], t[:])
                return s

            bq_s, bk_s, bv_s = load_pvec(bq), load_pvec(bk), load_pvec(bv)
            bproj_s, bfc_s, bmlp_s = load_pvec(bproj), load_pvec(bfc), load_pvec(bmlp)

            h2 = persist.tile([P, DC, Q], dt.float32, tag="h2")
            hq_sb = persist.tile([P, DC, Q], dt.float32, tag="hq")

            # LN stats for a [P, DC, KQ] fp32 block resident in SBUF.
            # sq on GpSimd, partition-sums via PE ones-matmuls, row math on
            # DVE, sqrt on scalar, reciprocal via fast DVE approx.
            def ln_stats(get_chunk, lnps, tag, rowtag="lnrow", sq_dve=False):
                pss = lnps.tile([1, KQ], dt.float32, tag=rowtag)
                psq = lnps.tile([1, KQ], dt.float32, tag=rowtag)
                eng = nc.vector if sq_dve else nc.gpsimd
                for c in range(DC):
                    xc = get_chunk(c)
                    sq = tmp.tile([P, KQ], dt.float32, tag="sq")
                    eng.tensor_tensor(sq[:], xc, xc, Alu.mult)
                    nc.tensor.matmul(pss[:], ones_col[:], xc,
                                     start=(c == 0), stop=(c == DC - 1))
                    nc.tensor.matmul(psq[:], ones_col[:], sq[:],
                                     start=(c == 0), stop=(c == DC - 1))
                mean = rows.tile([1, KQ], dt.float32, tag="mean")
                nc.vector.tensor_scalar_mul(mean[:], pss[:], 1.0 / D)
                msq = rowtmp.tile([1, KQ], dt.float32, tag="lnrow")
                nc.vector.tensor_tensor(msq[:], mean[:], mean[:], Alu.mult)
                var = rowtmp.tile([1, KQ], dt.float32, tag="lnrow")
                nc.vector.scalar_tensor_tensor(
                    var[:], psq[:], 1.0 / D, msq[:], Alu.mult, Alu.subtract
                )
                std = rowtmp.tile([1, KQ], dt.float32, tag="lnrow")
                nc.scalar.activation(std[:], var[:], F.Sqrt, bias=eps_t[:])
                rstd = rows.tile([1, KQ], dt.float32, tag="rstd")
                nc.vector.reciprocal_approx_fast(rstd[:], std[:])
                return mean, rstd

            # apply pass: xln = (x - mean) * rstd (gamma/beta folded into
            # the consuming weights host-side), written as `odt`.
            def ln_apply_start(mean, rstd, bbpool, bbtag):
                mb = bbpool.tile([P, KQ], dt.float32, tag=bbtag)
                rb = bbpool.tile([P, KQ], dt.float32, tag=bbtag)
                nc.tensor.matmul(mb[:], ones_row[:], mean[:], start=True, stop=True)
                nc.tensor.matmul(rb[:], ones_row[:], rstd[:], start=True, stop=True)
                return mb, rb

            def ln_apply_chunk(get_chunk, mb, rb, out, c, gp=False):
                eng = nc.gpsimd if gp else nc.vector
                xc = get_chunk(c)
                t1 = tmp.tile([P, KQ], dt.float32, tag="lnt1")
                eng.tensor_tensor(t1[:], xc, mb[:], Alu.subtract)
                eng.tensor_tensor(out[:, c, :], t1[:], rb[:], Alu.mult)

            def ln_apply(get_chunk, mean, rstd, out, bbpool, bbtag,
                         gp_chunks=0):
                mb, rb = ln_apply_start(mean, rstd, bbpool, bbtag)
                for c in range(DC):
                    ln_apply_chunk(get_chunk, mb, rb, out, c,
                                   gp=(c >= DC - gp_chunks))

            with (
                tc.tile_pool(name="attnsc", bufs=1) as attnsc,
                tc.tile_pool(name="hqp", bufs=2) as hqp,
                tc.tile_pool(name="xlnp", bufs=1) as xlnp,
                tc.tile_pool(name="wkv", bufs=4) as wkv,
                tc.tile_pool(name="wvp", bufs=2) as wvp,
                tc.tile_pool(name="etp", bufs=3) as etp,
                tc.tile_pool(name="scp", bufs=3, space="PSUM") as scp,
                tc.tile_pool(name="fillp", bufs=1, space="PSUM") as fillp,
                tc.tile_pool(name="pvps", bufs=2, space="PSUM") as pvps,
                tc.tile_pool(name="lnps", bufs=2, space="PSUM") as lnps,
            ):
                qt = attnsc.tile([P, DC, Q], adt, tag="qt")
                attn_acc = attnsc.tile([65, H, Q], dt.float32, tag="attn_acc")
                kt_sb = attnsc.tile([P, DC, KQ], adt, tag="kt")
                v_sb = attnsc.tile([P, NKT, H * 65], adt, tag="v")
                vview = v_sb[:].rearrange("p k (h x) -> p k h x", x=65)
                nc.vector.tensor_copy(
                    vview[:, :, :, 64:65],
                    ones_col[:].to_broadcast([P, NKT, H, 1]),
                )
                dmask_s = const.tile([P, NKT, P], adt, tag="dmask")
                nc.sync.dma_start(dmask_s[:], dmask[:])

                hquart = {}

                def load_quarter(q):
                    t = hqp.tile([P, DC, KQ], dt.float32, tag="hquart")
                    for c in range(DC):
                        nc.sync.dma_start(
                            t[:, c, :], hT_r[:, c, q * KQ:(q + 1) * KQ]
                        )
                    hquart[q] = t

                def hchunk(q):
                    return lambda c: hquart[q][:, c, :]

                load_quarter(3)
                stats = {3: ln_stats(hchunk(3), lnps, "3")}

                xln_t = {}

                def make_xln_start(qq):
                    xln = xlnp.tile([P, DC, KQ], adt, tag="xln")
                    xln_t[qq] = xln
                    mean, rstd = stats.pop(qq)
                    return mean, rstd, xln

                def g_k(xln, p, pool, tag, on_scalar=False):
                    wk_t = wkv.tile([P, DC, P], adt, tag="wkq")
                    nc.sync.dma_start(wk_t[:], w_k[p])
                    psk = pool.tile([P, KQ], dt.float32, tag=tag)
                    for c in range(DC):
                        nc.tensor.matmul(
                            psk[:], wk_t[:, c, :], xln[:, c, :],
                            start=(c == 0), stop=(c == DC - 1),
                        )
                        if c == DC - 1:
                            if on_scalar:
                                nc.scalar.activation(
                                    kt_sb[:, p, :], psk[:], F.Identity,
                                    bias=bk_s[:, p:p + 1],
                                )
                            else:
                                nc.vector.tensor_scalar_add(
                                    kt_sb[:, p, :], psk[:], bk_s[:, p:p + 1]
                                )
                        yield

                def g_v(xln, wv_t, vs, kt, pool, tag):
                    psv = pool.tile([P, KQ], dt.float32, tag=tag)
                    for c in range(DC):
                        nc.tensor.matmul(
                            psv[:], xln[:, c, kt * P:(kt + 1) * P],
                            wv_t[:, c, :],
                            start=(c == 0), stop=(c == DC - 1),
                        )
                        if c == DC - 1:
                            dst = v_sb[
                                :, kt, vs * 8 * 65:(vs + 1) * 8 * 65
                            ].rearrange("p (h x) -> p h x", x=65)[:, :, 0:64]
                            nc.vector.tensor_copy(
                                dst, psv[:].rearrange("p (h x) -> p h x", x=64)
                            )
                        yield

                def g_q(xln, q, p, pool, tag):
                    wq_t = wkv.tile([P, DC, P], adt, tag="wkq")
                    nc.sync.dma_start(wq_t[:], w_q[p])
                    psq_ = pool.tile([P, KQ], dt.float32, tag=tag)
                    for c in range(DC):
                        nc.tensor.matmul(
                            psq_[:, 0:P], wq_t[:, c, :], xln[:, c, 384:512],
                            start=(c == 0), stop=(c == DC - 1),
                        )
                        if c == DC - 1:
                            nc.vector.tensor_scalar_add(
                                qt[:, p, q * P:(q + 1) * P], psq_[:, 0:P],
                                bq_s[:, p:p + 1],
                            )
                        yield

                def run_all(gen):
                    for _ in gen:
                        pass

                attnT2 = attnsc.tile([P, DC, Q], pdt, tag="attnT2")

                def emit_norm(h):
                    # per-head softmax normalization, interleaved right
                    # after head h's last (q=0) PV accumulation
                    srow = rowtmp.tile([1, Q], dt.float32, tag="srow")
                    nc.vector.tensor_copy(srow[:], attn_acc[64:65, h, :])
                    rrow = rowtmp.tile([1, Q], dt.float32, tag="rrow")
                    nc.vector.reciprocal_approx_fast(rrow[:], srow[:])
                    bc = lnps.tile([P, Q], dt.float32, tag="lnrow")
                    nc.tensor.matmul(
                        bc[0:64, :], ones_row[0:1, 0:64], rrow[:],
                        start=True, stop=True,
                    )
                    t1 = tmp.tile([HD, Q], dt.float32, tag="anorm")
                    nc.vector.tensor_tensor(
                        t1[:], attn_acc[0:64, h, :], bc[0:64, :], Alu.mult
                    )
                    off = 64 * (h // 8)
                    nc.vector.tensor_scalar_add(
                        attnT2[off:off + 64, h % 8, :], t1[:], bv_s[:, h:h + 1]
                    )

                def emit_proj(mo):
                    wp_t = wkv.tile([P, DC, P], pdt, tag="wkq")
                    nc.sync.dma_start(wp_t[:], w_projr[mo])
                    psp = scp.tile([P, KQ], dt.float32, tag="sc")
                    for c in range(DC):
                        nc.tensor.matmul(
                            psp[:], wp_t[:, c, :], attnT2[:, c, :],
                            start=(c == 0), stop=(c == DC - 1),
                        )
                    nc.vector.scalar_tensor_tensor(
                        h2[:, mo, :], psp[:], bproj_s[:, mo:mo + 1],
                        hq_sb[:, mo, :], Alu.add, Alu.add,
                    )

                et_t = {}

                def emit_scores(q, h, NQ, qsl, pull):
                    hp, hs = h // 2, (h % 2) * 64
                    et = etp.tile([P, NKT, KQ], adt, tag="et")
                    et_t[h] = et
                    if NQ <= 256:
                        # [P, 2, NQ] fits one PSUM bank: pair the score MMs
                        # and halve the exp-ACT count
                        for half in range(2):
                            psc = scp.tile([P, 2, 256], dt.float32, tag="sc")
                            for k2 in range(2):
                                kt = half * 2 + k2
                                nc.tensor.matmul(
                                    psc[:, k2, 0:NQ],
                                    kt_sb[hs:hs + 64, hp,
                                          kt * P:(kt + 1) * P],
                                    qt[hs:hs + 64, hp, qsl],
                                    start=True, stop=True,
                                )
                                pull()
                            nc.scalar.activation(
                                et[:, half * 2:half * 2 + 2, 0:NQ],
                                psc[:, :, 0:NQ], F.Exp, scale=0.125,
                            )
                            nc.gpsimd.tensor_tensor(
                                et[:, half * 2:half * 2 + 2, 0:P],
                                et[:, half * 2:half * 2 + 2, 0:P],
                                dmask_s[:, half * 2:half * 2 + 2, :],
                                Alu.mult,
                            )
                            pull()
                            pull()
                        return
                    for kt in range(NKT):
                        psc = scp.tile([P, KQ], dt.float32, tag="sc")
                        nc.tensor.matmul(
                            psc[:, 0:NQ],
                            kt_sb[hs:hs + 64, hp, kt * P:(kt + 1) * P],
                            qt[hs:hs + 64, hp, qsl],
                            start=True, stop=True,
                        )
                        pull()
                        nc.scalar.activation(
                            et[:, kt, 0:NQ], psc[:, 0:NQ], F.Exp, scale=0.125,
                        )
                        if kt % 2 == 1:
                            # multiplicative causal mask on the diagonal
                            # query tile (first 128 columns of the window)
                            nc.gpsimd.tensor_tensor(
                                et[:, kt - 1:kt + 1, 0:P],
                                et[:, kt - 1:kt + 1, 0:P],
                                dmask_s[:, kt - 1:kt + 1, :], Alu.mult,
                            )
                        pull()

                def emit_pv(q, h, NQ, qsl):
                    et = et_t.pop(h)
                    pa = pvps.tile([65, KQ], dt.float32, tag="pv")
                    for kt in range(NKT):
                        nc.tensor.matmul(
                            pa[:, qsl], v_sb[:, kt, h * 65:h * 65 + 65],
                            et[:, kt, 0:NQ],
                            start=(kt == 0), stop=(kt == NKT - 1),
                        )
                    nc.scalar.activation(
                        attn_acc[:, h, q * P:(q + 1) * P],
                        pa[:, q * P:(q + 1) * P], F.Copy,
                    )
                    if q < NQT - 1:
                        nc.vector.tensor_tensor(
                            attn_acc[:, h, (q + 1) * P:],
                            attn_acc[:, h, (q + 1) * P:],
                            pa[:, (q + 1) * P:], Alu.add,
                        )
                    if q == 0:
                        emit_norm(h)

                # prologue: quarter 3 LN fully, eagerly; own-rows DMA for
                # the residual path is issued after the critical q3 data
                load_quarter(3)
                stats[3] = ln_stats(hchunk(3), pvps, "3", rowtag="pv", sq_dve=True)
                mean3, rstd3, xln3 = make_xln_start(3)
                mb3, rb3 = ln_apply_start(mean3, rstd3, lnps, "lnrow")
                for c in range(DC):
                    ln_apply_chunk(hchunk(3), mb3, rb3, xln3, c)

                for q in range(NQT - 1, -1, -1):
                    if q == 1:
                        # residual own-rows, needed only by the proj phase
                        for c in range(DC):
                            nc.sync.dma_start(
                                hq_sb[:, c, :],
                                hT_r[:, c, :].rearrange(
                                    "p (j n) -> p j n", n=KQ
                                )[:, :, 384:512],
                            )
                    xln = xln_t.pop(q)
                    nxt = None
                    if q > 0:
                        load_quarter(q - 1)
                        stats[q - 1] = ln_stats(hchunk(q - 1), pvps,
                                                str(q - 1), rowtag="pv")
                        nxt = make_xln_start(q - 1)

                    # preamble: K0, V(vs0) x4, Q0..Q3 dense (scp slots)
                    wv0 = wvp.tile([P, DC, 512], adt, tag="wv")
                    nc.sync.dma_start(wv0[:], w_v[0])
                    wv1 = wvp.tile([P, DC, 512], adt, tag="wv")
                    nc.sync.dma_start(wv1[:], w_v[1])
                    run_all(g_k(xln, 0, scp, "sc"))
                    for kt in range(NKT):
                        run_all(g_v(xln, wv0, 0, kt, scp, "sc"))
                    run_all(g_k(xln, 1, scp, "sc"))
                    for p in range(4):
                        run_all(g_q(xln, q, p, scp, "sc"))

                    # fine-grained fill queue: two MMs pulled after every
                    # score so the PE never drains while exp catches up
                    def fill_iter():
                        yield from g_k(xln, 2, fillp, "mm", on_scalar=(q > 0))
                        for kt in range(2):
                            yield from g_v(xln, wv1, 1, kt, fillp, "mm")
                        yield from g_k(xln, 3, fillp, "mm", on_scalar=(q > 0))
                        for kt in range(2, 4):
                            yield from g_v(xln, wv1, 1, kt, fillp, "mm")
                        yield from g_k(xln, 4, fillp, "mm", on_scalar=(q > 0))
                        yield from g_q(xln, q, 4, fillp, "mm")
                        yield from g_k(xln, 5, fillp, "mm", on_scalar=(q > 0))
                        yield from g_q(xln, q, 5, fillp, "mm")
                        yield from g_k(xln, 6, fillp, "mm", on_scalar=(q > 0))
                        yield from g_q(xln, q, 6, fillp, "mm")
                        yield from g_k(xln, 7, fillp, "mm", on_scalar=(q > 0))
                        yield from g_q(xln, q, 7, fillp, "mm")

                    fq = fill_iter()

                    def pull():
                        next(fq, None)

                    NQ = (NQT - q) * P
                    qsl = slice(q * P, Q)
                    for h in range(H):
                        if h == 0 and nxt is not None:
                            mb_n, rb_n = ln_apply_start(nxt[0], nxt[1],
                                                        lnps, "lnrow")
                        emit_scores(q, h, NQ, qsl, pull)
                        if h > 0:
                            emit_pv(q, h - 1, NQ, qsl)
                        if h % 2 == 1 and nxt is not None:
                            ln_apply_chunk(hchunk(q - 1), mb_n, rb_n,
                                           nxt[2], h // 2)
                    run_all(fq)
                    emit_pv(q, H - 1, NQ, qsl)

                if dbg:
                    nc.sync.dma_start(dbg_qt[:], qt[:])
                    nc.sync.dma_start(dbg_acc[:], attn_acc[:])

                for mo in range(DC):
                    emit_proj(mo)

                # LN2 stats (h2 complete after proj)
                pss2 = pvps.tile([1, KQ], dt.float32, tag="pv")
                psq2 = pvps.tile([1, KQ], dt.float32, tag="pv")
                for mo in range(DC):
                    sq2 = tmp.tile([P, KQ], dt.float32, tag="sq")
                    nc.gpsimd.tensor_tensor(
                        sq2[:], h2[:, mo, :], h2[:, mo, :], Alu.mult
                    )
                    nc.tensor.matmul(pss2[:], ones_col[:], h2[:, mo, :],
                                     start=(mo == 0), stop=(mo == DC - 1))
                    nc.tensor.matmul(psq2[:], ones_col[:], sq2[:],
                                     start=(mo == 0), stop=(mo == DC - 1))
                mean2 = rows.tile([1, KQ], dt.float32, tag="mean")
                nc.vector.tensor_scalar_mul(mean2[:], pss2[:], 1.0 / D)
                msq2 = rowtmp.tile([1, KQ], dt.float32, tag="lnrow")
                nc.vector.tensor_tensor(msq2[:], mean2[:], mean2[:], Alu.mult)
                var2 = rowtmp.tile([1, KQ], dt.float32, tag="lnrow")
                nc.vector.scalar_tensor_tensor(
                    var2[:], psq2[:], 1.0 / D, msq2[:], Alu.mult, Alu.subtract
                )
                std2 = rowtmp.tile([1, KQ], dt.float32, tag="lnrow")
                nc.scalar.activation(std2[:], var2[:], F.Sqrt, bias=eps_t[:])
                rstd2 = rows.tile([1, KQ], dt.float32, tag="rstd")
                nc.vector.reciprocal_approx_fast(rstd2[:], std2[:])
                if dbg:
                    nc.sync.dma_start(dbg_at2[:], attnT2[:])

            # ---- LN2 / fc+gelu / mlp + residual ----
            with (
                tc.tile_pool(name="mlpsc", bufs=1) as mlpsc,
                tc.tile_pool(name="wfcs", bufs=4) as wfcs,
                tc.tile_pool(name="wmlps", bufs=4) as wmlps,
                tc.tile_pool(name="psfc", bufs=2, space="PSUM") as psfc,
                tc.tile_pool(name="psm", bufs=2, space="PSUM") as psm,
                tc.tile_pool(name="lnps2", bufs=2, space="PSUM") as lnps2,
            ):
                h2c = lambda c: h2[:, c, :]
                h2n = mlpsc.tile([P, DC, Q], pdt, tag="h2n")
                ln_apply(h2c, mean2, rstd2, h2n, lnps2, "lnbb")
                if dbg:
                    nc.sync.dma_start(dbg_h2[:], h2[:])
                    nc.sync.dma_start(dbg_h2n[:], h2n[:])
                y2 = mlpsc.tile([P, DC, Q], dt.float32, tag="y2")
                g_half = mlpsc.tile([P, IC // 2, Q], pdt, tag="g")
                for ih in range(2):
                    for m in range(IC // 2):
                        mg = ih * (IC // 2) + m
                        wfc_t = wfcs.tile([P, DC, P], pdt, tag="wfc")
                        nc.sync.dma_start(wfc_t[:], w_fcr[mg])
                        psf = psfc.tile([P, Q], dt.float32, tag="fc")
                        for c in range(DC):
                            nc.tensor.matmul(
                                psf[:], wfc_t[:, c, :], h2n[:, c, :],
                                start=(c == 0), stop=(c == DC - 1),
                            )
                        nc.scalar.activation(
                            g_half[:, m, :], psf[:], F.Gelu,
                            bias=bfc_s[:, mg:mg + 1],
                        )
                    if dbg and ih == 0:
                        nc.sync.dma_start(dbg_g[:], g_half[:])
                    for mo in range(DC):
                        wm_t = wmlps.tile([P, IC // 2, P], pdt, tag="wmlp")
                        nc.sync.dma_start(wm_t[:], w_mlpr[ih, mo])
                        psm_ = psm.tile([P, Q], dt.float32, tag="mm2")
                        for c in range(IC // 2):
                            nc.tensor.matmul(
                                psm_[:], wm_t[:, c, :], g_half[:, c, :],
                                start=(c == 0), stop=(c == IC // 2 - 1),
                            )
                        if ih == 0:
                            nc.vector.tensor_copy(y2[:, mo, :], psm_[:])
                        else:
                            ot = tmp.tile([P, Q], dt.float32, tag="anorm")
                            nc.vector.tensor_tensor(
                                ot[:], y2[:, mo, :], psm_[:], Alu.add
                            )
                            nc.vector.scalar_tensor_tensor(
                                ot[:], ot[:], bmlp_s[:, mo:mo + 1],
                                h2[:, mo, :], Alu.add, Alu.add,
                            )
                            nc.sync.dma_start(
                                outT.rearrange("(c p) n -> p c n", p=P)[:, mo, :],
                                ot[:],
                            )

    nc.compile()
    return nc


def _get_nc(cfg):
    if cfg not in _BUILD_CACHE:
        _BUILD_CACHE[cfg] = _build(cfg)
    return _BUILD_CACHE[cfg]


def _np_dt(name):
    if name == "bf16":
        import ml_dtypes
        return ml_dtypes.bfloat16
    if name == "f16":
        return np.float16
    return np.float32


def _perm_for_core(a):
    """Key order per quarter: non-own keys in natural order, own 128 last."""
    perm = []
    for q in range(NQT):
        base = q * KQ
        own = np.arange(base + 128 * a, base + 128 * a + P)
        others = np.setdiff1d(np.arange(base, base + KQ), own)
        perm.append(np.concatenate([others, own]))
    return np.concatenate(perm)


def _prep_in_maps(inputs, cfg):
    adt_np, pdt_np = _np_dt(cfg[0]), _np_dt(cfg[1])
    h = np.asarray(inputs["hidden_states"], dtype=np.float32)
    w_qkv = np.asarray(inputs["w_qkv"], np.float32)
    b_qkv = np.asarray(inputs["b_qkv"], np.float32)
    g1 = np.asarray(inputs["g1"], np.float32)
    be1 = np.asarray(inputs["be1"], np.float32)
    g2 = np.asarray(inputs["g2"], np.float32)
    be2 = np.asarray(inputs["be2"], np.float32)

    # fold LN1 gamma/beta into the qkv weights and biases
    w_qkv_f = w_qkv * g1[:, None]
    b_qkv_f = b_qkv + be1 @ w_qkv
    w_fc = np.asarray(inputs["w_fc"], np.float32)
    b_fc = np.asarray(inputs["b_fc"], np.float32)
    w_fc_f = w_fc * g2[:, None]
    b_fc_f = b_fc + be2 @ w_fc

    def chunk_w(w, p=P):  # [Din, N] -> [p, Din//p, N]
        return np.ascontiguousarray(w.reshape(-1, p, w.shape[1]).transpose(1, 0, 2))

    def pvec(v, p=P):  # [n*p] -> [p, n]
        return np.ascontiguousarray(v.reshape(-1, p).T)

    def mslice(a, nsl):  # [p, c, n] -> [n//nsl, p, c, nsl]
        p, c, n = a.shape
        return np.ascontiguousarray(
            a.reshape(p, c, n // nsl, nsl).transpose(2, 0, 1, 3)
        )

    wq = mslice(chunk_w(w_qkv_f[:, 0:D]), P)
    wk = mslice(chunk_w(w_qkv_f[:, D:2 * D]), P)
    wv = mslice(chunk_w(w_qkv_f[:, 2 * D:3 * D]), 512)

    # proj with head pairs (c, c+8) stacked on the 128 contraction rows
    w_proj = np.asarray(inputs["w_proj"], np.float32)
    w2 = w_proj.reshape(H, HD, D)
    wp_t = np.concatenate([w2[0:8], w2[8:16]], axis=1)  # [8, 128, 1024]
    wp = np.ascontiguousarray(
        wp_t.reshape(DC, P, DC, P).transpose(2, 1, 0, 3)
    )

    wfc = mslice(chunk_w(w_fc_f), P)
    wm = chunk_w(np.asarray(inputs["w_mlp"], np.float32))  # [128, 32, 1024]
    wmlp = np.ascontiguousarray(
        wm.reshape(P, 2, IC // 2, DC, P).transpose(1, 3, 0, 2, 4)
    )

    shared = {
        "w_q": wq.astype(adt_np), "w_k": wk.astype(adt_np),
        "w_v": wv.astype(adt_np), "w_projr": wp.astype(pdt_np),
        "w_fcr": wfc.astype(pdt_np), "w_mlpr": wmlp.astype(pdt_np),
        "bq": pvec(b_qkv_f[0:D]),
        "bk": pvec(b_qkv_f[D:2 * D]),
        "bv": pvec(b_qkv_f[2 * D:3 * D], p=HD),
        "bproj": pvec(np.asarray(inputs["b_proj"], np.float32)),
        "bfc": pvec(b_fc_f),
        "bmlp": pvec(np.asarray(inputs["b_mlp"], np.float32)),
    }
    in_maps = []
    for core in range(8):
        b, a = core // 4, core % 4
        perm = _perm_for_core(a)
        hrot = h[b, perm]
        # multiplicative 0/1 mask on exp'd scores for the diagonal query
        # tile: key position j=kt*128+p in the quarter vs query row r.
        pp = np.arange(P)
        dm = np.zeros((P, NKT, P), np.float32)
        for kt in range(NKT - 1):
            dm[:, kt, :] = ((kt * P + pp) < (P * a)).astype(np.float32)[:, None]
        dm[:, NKT - 1, :] = (pp[:, None] <= pp[None, :]).astype(np.float32)
        in_maps.append(
            dict(
                shared,
                hT=np.ascontiguousarray(hrot.T),
                dmask=dm.astype(adt_np),
            )
        )
    return in_maps


def _stitch(results):
    out = np.empty((2, S, D), dtype=np.float32)
    for core in range(8):
        b, a = core // 4, core % 4
        r = results[core]["outT"].T  # [512, D]: cols j*128+p -> row 512j+128a+p
        for j in range(NQT):
            out[b, j * KQ + P * a: j * KQ + P * a + P] = r[j * P:(j + 1) * P]
    return out


def run(inputs, cfg=("bf16", "bf16"), trace=False, trace_cores=None):
    nc = _get_nc(cfg)
    in_maps = _prep_in_maps(inputs, cfg)
    res = bass_utils.run_bass_kernel_spmd(
        nc, in_maps, core_ids=list(range(8)), trace=trace, trace_cores=trace_cores
    )
    return _stitch(res.results), res


def kernel(**inputs) -> np.ndarray:
    out, _ = run(inputs, cfg=("bf16", "bf16"))
    return out
